# revision 5
# baseline (speedup 1.0000x reference)
"""Trainium2 Bass kernel for nn_CustomCLIP_11407433138213 (moe_routing).

Math (per sample b with domain n = labels[b]):
    h   = relu(x @ W1[n])                 [R]
    a   = relu(h @ W2[n])                 [D]
    f   = 0.2*a + 0.8*x                   [D]
    out = exp(ls) * (f/||f||) @ T^T       [N_TXT]

Strategy (v2): data-parallel over batch, 8 cores x 2048 rows, host sorts
rows by domain so each 512-row block is single-domain (mixed-block
minority rows exactly recomputed on host and patched in, as before).

Numerics/layout changes vs v1 (all validated by direct HW probes):
  - All DRAM traffic is bf16 or fp8 payloads packed into f32-declared
    DRAM tensors (2-byte/1-byte dtypes get corrupted by the DRAM param
    binding; bitcast on the SBUF side is fine). Cuts DMA 33MB -> 17MB
    per core.
  - mm1/mm2 run in fp8e4 (e4m3) with MatmulPerfMode.DoubleRow: K=256
    per pass at 0.5 cyc/row -> 4x fewer PE cycles than f32r. Scales are
    folded so everything stays exact powers of two:
        xb    = bf16(32*x)         (the ft addend; also fp8 copy x8)
        W1_8  = e4m3(8*W1), g = relu(ph)*2^-8 = h   (fp8)
        W2_8  = e4m3(8*W2), pa = 8*(h@W2), ft = relu(pa)+xb = 32*f/0.8
    The norm divides the 40x scale back out, so logits are unchanged.
  - mm3 stays 1 cyc/row with bf16 ft x bf16 text tiles.
  - Output written as engine-rounded bf16 (RNE), packed into f32 DRAM.
  Measured end-to-end rel err (CPU sim): ~3.6e-3 vs 2e-2 budget.

Per row-block of 512 (samples on the moving/free dim):
    - mm1: 2 m-tiles x 4 K-pairs fp8-DR, relu*2^-8 -> g fp8 [128,2,512]
    - mm2: 8 d-chunks x 1 fp8-DR matmul; ft = relu(pa)+xb (DVE, bf16)
    - norm: s = colsum(ft^2) via ones-matmul; iv = rsqrt(s*exp(-2ls))
    - mm3: 11 t-tiles x 8 K-chunks bf16; ob = pl * bcast(iv) -> bf16
"""

import contextlib
import sys

sys.path.insert(0, "/opt/trn_rl_repo")

import ml_dtypes
import numpy as np

import concourse.bass as bass  # noqa: F401  (registers engine types)
import concourse.mybir as mybir
import concourse.tile as tile
from concourse import bacc
from concourse.bass_utils import run_bass_kernel_spmd

# Problem constants (hardcoded per task contract).
B, D, R, ND, NT = 16384, 1024, 256, 3, 1380
NC = 8                    # cores
BPC = B // NC             # rows per core = 2048
RB = 512                  # row-block (matmul moving dim)
NB = BPC // RB            # row-blocks per core = 4
NBLK = B // RB            # 32 global blocks
KD = D // 128             # 8 contraction chunks over D
KJ = KD // 2              # 4 DoubleRow K-pairs over D
MR = R // 128             # 2 M-chunks over R
NTP = 1408                # text padded to 11*128
TTI = NTP // 128          # 11 text chunks

F32 = mybir.dt.float32
F32R = mybir.dt.float32r
BF16 = mybir.dt.bfloat16
FP8 = mybir.dt.float8e4
DRM = mybir.MatmulPerfMode.DoubleRow

BF = ml_dtypes.bfloat16
E4 = ml_dtypes.float8_e4m3


def build_program():
    nc = bacc.Bacc(
        "TRN2",
        target_bir_lowering=False,
        debug=False,
        enable_asserts=True,
        num_devices=NC,
    )
    # All inputs are packed payloads in f32-declared DRAM tensors.
    xb_pk = nc.declare_dram_parameter("xb_pk", [D, BPC // 2], F32, isOutput=False)
    # fp8 activations are DMA'd for blocks 0-1 (startup critical path);
    # blocks 2-3 derive them on-device from the bf16 xb tiles via Act
    # copies, cutting DMA in the startup-critical window
    # 2-D block-major layouts: every load is a single contiguous run per
    # partition row (1KB for x8, 512B w1, 2KB w2) -- descriptor-count
    # overhead penalizes sub-1KB multi-run DMAs (measured on outputs)
    x8_pk = nc.declare_dram_parameter(
        "x8_pk", [2 * KJ * 128, 2 * RB // 4], F32, isOutput=False
    )
    w1_pk = nc.declare_dram_parameter(
        "w1_pk", [NB * KJ * 128, 2 * R // 4], F32, isOutput=False
    )
    w2_pk = nc.declare_dram_parameter(
        "w2_pk", [NB * 128, 2 * D // 4], F32, isOutput=False
    )
    tt_pk = nc.declare_dram_parameter("tt_pk", [D, NTP // 2], F32, isOutput=False)
    # NOTE: no small const params (sc/oc/orow). Binding 8 cores with tiny
    # ([1,1]/[128,1]) params alongside the big packed tensors corrupts the
    # big tensors' payloads from byte 2048 on (tf32-style mantissa damage,
    # isolated by probe bisection). The ones vectors are memset on device
    # and exp(logit_scale) is folded into the host-side bf16 text matrix.
    ot = nc.declare_dram_parameter("ot", [NTP, BPC // 2], F32, isOutput=True)
    import os

    DBG = os.environ.get("KDBG") == "1"
    if DBG:
        # all debug outputs padded +4 cols so the DRAM AP never collapses
        # to a fully-contiguous 1-D run (which corrupts past 4 partitions)
        d_ph = nc.declare_dram_parameter("d_ph", [128, RB + 4], F32, isOutput=True)
        d_g = nc.declare_dram_parameter("d_g", [128, MR * RB // 4 + 4], F32, isOutput=True)
        d_pa = nc.declare_dram_parameter("d_pa", [128, RB + 4], F32, isOutput=True)
        d_ft = nc.declare_dram_parameter("d_ft", [128, RB // 2 + 4], F32, isOutput=True)
        d_acc = nc.declare_dram_parameter("d_acc", [128, RB + 4], F32, isOutput=True)
        d_iv = nc.declare_dram_parameter("d_iv", [1, RB], F32R, isOutput=True)
        d_pl = nc.declare_dram_parameter("d_pl", [128, RB + 4], F32, isOutput=True)
        d_w1 = nc.declare_dram_parameter("d_w1", [128, R // 2 + 4], F32, isOutput=True)
        d_x8 = nc.declare_dram_parameter("d_x8", [128, RB // 2 + 4], F32, isOutput=True)
        d_w1b = nc.declare_dram_parameter("d_w1b", [128, R // 2 + 4], F32, isOutput=True)

    with tile.TileContext(nc) as tc, contextlib.ExitStack() as ctx:
        cst = ctx.enter_context(tc.tile_pool(name="cst", bufs=1))
        p_w1 = ctx.enter_context(tc.tile_pool(name="p_w1", bufs=NB * KJ))
        p_w2 = ctx.enter_context(tc.tile_pool(name="p_w2", bufs=NB))
        p_x8 = ctx.enter_context(tc.tile_pool(name="p_x8", bufs=NB * KJ))
        p_xb = ctx.enter_context(tc.tile_pool(name="p_xb", bufs=2 * KD))
        p_g = ctx.enter_context(tc.tile_pool(name="p_g", bufs=2))
        p_fp = ctx.enter_context(tc.tile_pool(name="p_fp", bufs=2 * KD))
        p_sq = ctx.enter_context(tc.tile_pool(name="p_sq", bufs=2 * KD))
        p_pbs = ctx.enter_context(tc.tile_pool(name="p_pbs", bufs=2))
        p_ob = ctx.enter_context(tc.tile_pool(name="p_ob", bufs=3))
        p_nrm = ctx.enter_context(tc.tile_pool(name="p_nrm", bufs=2))

        ps_h = ctx.enter_context(tc.tile_pool(name="ps_h", bufs=2, space="PSUM"))
        ps_a = ctx.enter_context(tc.tile_pool(name="ps_a", bufs=2, space="PSUM"))
        ps_s = ctx.enter_context(tc.tile_pool(name="ps_s", bufs=1, space="PSUM"))
        ps_l = ctx.enter_context(tc.tile_pool(name="ps_l", bufs=3, space="PSUM"))

        # ---- constant tiles -------------------------------------------
        ttt = [
            cst.tile([128, NTP], BF16, name=f"tt_{k}", tag=f"tt_{k}")
            for k in range(KD)
        ]
        # memset only supports plain dtypes; bitcast to F32R at the matmul
        ones_col = cst.tile([128, 1], F32, name="ones_col", tag="ones_col")
        ones_row = cst.tile([1, 128], F32, name="ones_row", tag="ones_row")

        # per-block live tiles
        S = [dict() for _ in range(NB)]

        def emit_consts():
            nc.any.memset(ones_col[:], 1.0)
            nc.any.memset(ones_row[:], 1.0)

        def emit_warmup():
            # dummy matmuls during the initial DMA wait so the PE clock
            # is fully ramped when the real work arrives
            wrm = cst.tile([128, RB], F32, name="wrm", tag="wrm")
            nc.gpsimd.memset(wrm[:], 0.0)
            for i in range(6):
                pw = ps_l.tile([1, RB], F32, name="pl", tag="pl")
                nc.tensor.matmul(
                    pw[:], ones_col[:].bitcast(F32R), wrm[:].bitcast(F32R),
                    start=True, stop=True,
                )

        def emit_tt(k):
            # one full load per chunk: 2816B contiguous rows (descriptor
            # overhead penalizes splitting)
            nc.sync.dma_start(
                ttt[k][:].bitcast(F32), tt_pk[k * 128 : (k + 1) * 128, :]
            )

        def emit_wxa(b):
            # critical path for mm1(b): expert W1 (+ fp8 acts for blocks 0-1)
            w1t = []
            for j in range(KJ):
                w = p_w1.tile([128, 2, R], FP8, name="w1", tag="w1")
                nc.sync.dma_start(
                    w[:].bitcast(F32).rearrange("p a b -> p (a b)"),
                    w1_pk[(b * KJ + j) * 128 : (b * KJ + j + 1) * 128, :],
                )
                w1t.append(w)
            S[b]["w1t"] = w1t
            if b < 2:
                x8t = []
                for j in range(KJ):
                    t = p_x8.tile([128, 2, RB], FP8, name="x8", tag="x8")
                    nc.sync.dma_start(
                        t[:].bitcast(F32).rearrange("p a b -> p (a b)"),
                        x8_pk[(b * KJ + j) * 128 : (b * KJ + j + 1) * 128, :],
                    )
                    x8t.append(t)
                S[b]["x8t"] = x8t

        def emit_x8conv(b):
            # blocks 2-3: quantize the bf16 xb tiles to fp8 on the Act
            # engine (d-chunk k = 2j+i maps to x8 tile j, half i)
            xb = S[b]["xb"]
            x8t = []
            for j in range(KJ):
                t = p_x8.tile([128, 2, RB], FP8, name="x8", tag="x8")
                for i in range(2):
                    nc.scalar.copy(t[:, i, :], xb[2 * j + i])
                x8t.append(t)
            S[b]["x8t"] = x8t

        def emit_w2(b):
            w2t = p_w2.tile([128, 2, D], FP8, name="w2", tag="w2")
            nc.sync.dma_start(
                w2t[:].bitcast(F32).rearrange("p a b -> p (a b)"),
                w2_pk[b * 128 : (b + 1) * 128, :],
            )
            S[b]["w2t"] = w2t

        def emit_xbp(pair):
            # bf16 activations for a BLOCK PAIR in one load per chunk:
            # the two blocks' samples are adjacent in xb_pk rows, so this
            # gives 2KB contiguous runs (vs 1KB) and halves the DMA count
            # on the largest input tensor
            c0 = 2 * pair * RB
            xb = []
            for k in range(KD):
                t = p_xb.tile([128, 2 * RB], BF16, name="xb", tag="xb")
                nc.sync.dma_start(
                    t[:].bitcast(F32),
                    xb_pk[k * 128 : (k + 1) * 128, c0 // 2 : (c0 + 2 * RB) // 2],
                )
                xb.append(t)
            for b in (2 * pair, 2 * pair + 1):
                off = (b % 2) * RB
                S[b]["xb"] = [t[:, off : off + RB] for t in xb]

        def emit_mm1_g(b):
            w1t, x8t = S[b]["w1t"], S[b]["x8t"]
            if DBG and b == 0:
                nc.sync.dma_start(
                    d_w1[:, : R // 2],
                    w1t[0][:].bitcast(F32).rearrange("p a b -> p (a b)"),
                )
                nc.sync.dma_start(
                    d_x8[:, : RB // 2],
                    x8t[0][:].bitcast(F32).rearrange("p a b -> p (a b)"),
                )
                tf = p_sq.tile([128, 128], F32, name="dw1b", tag="sq")
                nc.sync.dma_start(
                    tf[:], w1_pk[0:128, :, :].rearrange("p a b -> p (a b)")
                )
                nc.sync.dma_start(d_w1b[:, :128], tf[:])
            gt = p_g.tile([128, MR, RB], FP8, name="g", tag="g")
            for m in range(MR):
                ph = ps_h.tile([128, RB], F32, name="ph", tag="ph")
                for j in range(KJ):
                    nc.tensor.matmul(
                        ph[:],
                        w1t[j][:, :, m * 128 : (m + 1) * 128],
                        x8t[j][:, :, :],
                        start=(j == 0),
                        stop=(j == KJ - 1),
                        perf_mode=DRM,
                    )
                if DBG and b == 0 and m == 0:
                    dt_ = p_sq.tile([128, RB], F32, name="dph", tag="sq")
                    nc.scalar.copy(dt_[:], ph[:])
                    nc.sync.dma_start(d_ph[:, :RB], dt_[:])
                nc.scalar.activation(
                    gt[:, m, :],
                    ph[:],
                    mybir.ActivationFunctionType.Relu,
                    scale=2.0**-8,
                )
            if DBG and b == 0:
                nc.sync.dma_start(d_g[:, : MR * RB // 4], gt[:].bitcast(F32).rearrange("p a b -> p (a b)"))
            S[b]["g"] = gt

        def emit_mm2(b, d_lo=0, d_hi=KD):
            xb, gt, w2t = S[b]["xb"], S[b]["g"], S[b]["w2t"]
            if d_lo == 0:
                S[b]["fp"] = []
                S[b]["sq"] = []
            fp = S[b]["fp"]
            sq = S[b]["sq"]
            for d in range(d_lo, d_hi):
                pa = ps_a.tile([128, RB], F32, name="pa", tag="pa")
                nc.tensor.matmul(
                    pa[:],
                    w2t[:, :, d * 128 : (d + 1) * 128],
                    gt[:, :, :],
                    start=True,
                    stop=True,
                    perf_mode=DRM,
                )
                if DBG and b == 0 and d == 0:
                    dt_ = p_sq.tile([128, RB], F32, name="dpa", tag="sq")
                    nc.scalar.copy(dt_[:], pa[:])
                    nc.sync.dma_start(d_pa[:, :RB], dt_[:])
                ft = p_fp.tile([128, RB], BF16, name="fp", tag="fp")
                nc.vector.scalar_tensor_tensor(
                    ft[:],
                    pa[:],
                    0.0,
                    xb[d],
                    mybir.AluOpType.max,
                    mybir.AluOpType.add,
                )
                if DBG and b == 0 and d == 0:
                    nc.sync.dma_start(d_ft[:, : RB // 2], ft[:].bitcast(F32))
                fp.append(ft)
                # f32r squares: moving side of the accumulating
                # ones-matmul group in emit_ps_norm
                st = p_sq.tile([128, RB], F32R, name="sq", tag="sq")
                nc.scalar.square(st[:], ft[:])
                sq.append(st)

        def emit_ps_norm(b):
            # colsum of the 8 squares via one accumulating PE matmul group
            sq = S[b]["sq"]
            ps = ps_s.tile([1, RB], F32, name="ps", tag="ps")
            for d in range(KD):
                nc.tensor.matmul(
                    ps[:],
                    ones_col[:].bitcast(F32R),
                    sq[d][:],
                    start=(d == 0),
                    stop=(d == KD - 1),
                )
            iv = p_nrm.tile([1, RB], F32R, name="iv", tag="iv")
            nc.scalar.activation(
                iv[:],
                ps[:],
                mybir.ActivationFunctionType.Abs_reciprocal_sqrt,
            )
            if DBG and b == 0:
                nc.sync.dma_start(d_iv[:], iv[:])
            S[b]["iv"] = iv

        def emit_pb(b):
            iv = S[b]["iv"]
            pb = ps_l.tile([128, RB], F32, name="pl", tag="pl")
            nc.tensor.matmul(
                pb[:], ones_row[:].bitcast(F32R), iv[:], start=True, stop=True
            )
            pbs = p_pbs.tile([128, RB], F32, name="pbs", tag="pbs")
            nc.scalar.copy(pbs[:], pb[:])
            S[b]["pbs"] = pbs

        def emit_mm3_mm(b, t_i):
            fp = S[b]["fp"]
            pl = ps_l.tile([128, RB], F32, name="pl", tag="pl")
            for k in range(KD):
                nc.tensor.matmul(
                    pl[:],
                    ttt[k][:, t_i * 128 : (t_i + 1) * 128],
                    fp[k][:],
                    start=(k == 0),
                    stop=(k == KD - 1),
                )
            return pl

        def emit_mm3_ob(b, t_i, pl):
            c0 = b * RB
            ob = p_ob.tile([128, RB], BF16, name="ob", tag="ob")
            nc.vector.tensor_mul(ob[:], pl[:], S[b]["pbs"][:])
            # single write per tile: 1KB contiguous rows minimize DMA
            # descriptor overhead (finer splits measured strictly worse:
            # 2-way ~140us, 4-way last block 151-175us)
            nc.sync.dma_start(
                ot[t_i * 128 : (t_i + 1) * 128, c0 // 2 : (c0 + RB) // 2],
                ob[:].bitcast(F32),
            )

        def emit_mm3(b, t_lo, t_hi):
            for t_i in range(t_lo, t_hi):
                pl = emit_mm3_mm(b, t_i)
                if DBG and b == 0 and t_i == 0:
                    dt_ = p_nrm.tile([128, RB], F32, name="dpl", tag="dpl")
                    nc.scalar.copy(dt_[:], pl[:])
                    nc.sync.dma_start(d_pl[:, :RB], dt_[:])
                emit_mm3_ob(b, t_i, pl)
            if t_hi == TTI:
                S[b].clear()

        # ---- emission schedule (software pipelined) ---------------------
        # DMA priority order: block-0 criticals (w1/x8 then w2/xb), then
        # the tt stream (needed by mm3(0) at ~19us), then block 1, 2, 3.
        # PE order per iteration: mm1(b+1), pb(b), mm2(b+1), mm3(b),
        # ones(b+1) -- mm1(b+1) only needs the small w1/x8 tiles, which
        # land a full block period ahead.
        # PE stream ordered by operand readiness; mm1/mm2 of the next
        # block sit INSIDE mm3(b) so their DMA/Act dependencies have a
        # full mm3 window to land, and nothing ready ever queues behind
        # a stalled PE instruction. Per iteration:
        #   t0(b) | ones(b) | t1 | pb(b) | t2 t3 | mm1(b+1) | t4 |
        #   mm2(b+1) | t5..t10
        # DMA priority: block-0 criticals, w1(1), the tt stream (needed
        # by t0(0) at ~23us), then the rest block by block.
        emit_consts()
        emit_wxa(0)
        emit_w2(0)
        emit_xbp(0)
        for k in range(KD):
            emit_tt(k)
        emit_wxa(1)
        emit_w2(1)
        emit_warmup()
        emit_mm1_g(0)
        emit_mm2(0)
        for b in range(NB):
            if b + 2 < NB:
                emit_wxa(b + 2)
                emit_w2(b + 2)
                if b == 0:
                    emit_xbp(1)
            emit_ps_norm(b)
            if b + 1 < NB:
                emit_mm1_g(b + 1)
            pl0 = emit_mm3_mm(b, 0)
            emit_pb(b)
            emit_mm3_ob(b, 0, pl0)
            emit_mm3(b, 1, 3)
            if b + 1 < NB:
                emit_mm2(b + 1, 0, 4)
            emit_mm3(b, 3, 6)
            if b + 1 < NB:
                emit_mm2(b + 1, 4, KD)
            emit_mm3(b, 6, TTI)
            if b + 2 < NB:
                emit_x8conv(b + 2)

    nc.compile()
    return nc


_NC_CACHE = None


def _get_program():
    global _NC_CACHE
    if _NC_CACHE is None:
        _NC_CACHE = build_program()
    return _NC_CACHE


def _prep(image_features, domain_labels, W1, W2, text_features, logit_scale):
    """Host-side: sort rows by domain, quantize/pack per-core inputs,
    compute exact logits for minority rows of mixed blocks."""
    x = np.asarray(image_features, dtype=np.float32)
    labels = np.asarray(domain_labels).astype(np.int64)
    W1 = np.asarray(W1, dtype=np.float32)
    W2 = np.asarray(W2, dtype=np.float32)
    T = np.asarray(text_features, dtype=np.float32)
    ls = float(np.asarray(logit_scale))

    perm = np.argsort(labels, kind="stable")
    lab_s = labels[perm]
    blk_dom = np.empty(NBLK, dtype=np.int64)
    fix_sorted = np.zeros(B, dtype=bool)
    for i in range(NBLK):
        seg = lab_s[i * RB : (i + 1) * RB]
        counts = np.bincount(seg, minlength=ND)
        m = int(counts.argmax())
        blk_dom[i] = m
        if counts[m] != RB:
            fix_sorted[i * RB : (i + 1) * RB] = seg != m

    xs32T = np.ascontiguousarray((32.0 * x[perm]).T)        # [D, B] f32
    xb_bf = xs32T.astype(BF)                                # [D, B] bf16
    x8 = xs32T.astype(E4)                                   # [D, B] fp8
    # [j, p, i, B] with d = (2j+i)*128 + p
    x8r = np.ascontiguousarray(
        x8.reshape(KJ, 2, 128, B).transpose(0, 2, 1, 3)
    )
    W18 = (8.0 * W1).astype(E4)                             # [ND, D, R]
    W28 = (8.0 * W2).astype(E4)                             # [ND, R, D]
    # [n, j, p, i, m] with d = (2j+i)*128 + p
    w1r = np.ascontiguousarray(
        W18.reshape(ND, KJ, 2, 128, R).transpose(0, 1, 3, 2, 4)
    )
    # [n, p, i, d] with r = i*128 + p
    w2r = np.ascontiguousarray(
        W28.reshape(ND, 2, 128, D).transpose(0, 2, 1, 3)
    )
    # exp(ls) folded into the text matrix: logits = (e^ls T)^T ft / ||ft||
    ttp = np.zeros((D, NTP), dtype=np.float32)
    ttp[:, :NT] = np.exp(ls) * T.T
    tt_pk = np.ascontiguousarray(ttp.astype(BF)).view(np.float32)

    in_maps = []
    for c in range(NC):
        cols = slice(c * BPC, (c + 1) * BPC)
        doms = blk_dom[c * NB : (c + 1) * NB]
        xb_pk = np.ascontiguousarray(xb_bf[:, cols]).view(np.float32)
        # fp8 activations shipped for blocks 0-1 (block-major, so every
        # load is one contiguous 1KB run per row); blocks 2-3 quantize
        # on-device from xb
        x8_pk = (
            np.ascontiguousarray(
                x8r[:, :, :, c * BPC : c * BPC + 2 * RB]
                .reshape(KJ, 128, 2, 2, RB)
                .transpose(3, 0, 1, 2, 4)
            )
            .reshape(2 * KJ * 128, 2 * RB)
            .view(np.float32)
        )
        w1_pk = (
            np.ascontiguousarray(w1r[doms])
            .reshape(NB * KJ * 128, 2 * R)
            .view(np.float32)
        )
        w2_pk = (
            np.ascontiguousarray(w2r[doms])
            .reshape(NB * 128, 2 * D)
            .view(np.float32)
        )
        in_maps.append(
            {
                "xb_pk": xb_pk,
                "x8_pk": x8_pk,
                "w1_pk": w1_pk,
                "w2_pk": w2_pk,
                "tt_pk": tt_pk,
            }
        )

    # exact recompute for minority rows of mixed blocks
    fix_orig = perm[fix_sorted]
    fixed = np.empty((fix_orig.size, NT), dtype=np.float32)
    if fix_orig.size:
        xe = x[fix_orig]
        le = labels[fix_orig]
        for dcur in range(ND):
            m = le == dcur
            if not m.any():
                continue
            xm = xe[m]
            h = np.maximum(xm @ W1[dcur], 0.0)
            a = np.maximum(h @ W2[dcur], 0.0)
            f = 0.2 * a + 0.8 * xm
            f /= np.linalg.norm(f, axis=1, keepdims=True)
            fixed[m] = np.exp(ls) * (f @ T.T)
    return in_maps, perm, fix_orig, fixed


def make_in_maps(image_features, domain_labels, W1, W2, text_features, logit_scale):
    in_maps, _, _, _ = _prep(
        image_features, domain_labels, W1, W2, text_features, logit_scale
    )
    return in_maps


def kernel(image_features, domain_labels, W1, W2, text_features, logit_scale, **kw):
    in_maps, perm, fix_orig, fixed = _prep(
        image_features, domain_labels, W1, W2, text_features, logit_scale
    )
    nc = _get_program()
    res = run_bass_kernel_spmd(nc, in_maps, list(range(NC)))

    out_sorted = np.empty((B, NT), dtype=np.float32)
    for c in range(NC):
        ob = res.results[c]["ot"].view(BF)          # [NTP, BPC] bf16
        out_sorted[c * BPC : (c + 1) * BPC, :] = (
            ob[:NT, :].T.astype(np.float32)
        )
    out = np.empty((B, NT), dtype=np.float32)
    out[perm] = out_sorted
    if fix_orig.size:
        out[fix_orig] = fixed
    return out
